# revision 1
# baseline (speedup 1.0000x reference)
"""Trainium2 Bass kernel for nn_Network_5772436046487 (gnn_message_passing).

Recurrence (T=50 steps, B=8, N=50000 nodes, E=1.6M edges):
    v' = v + DT*(-v + bias + scatter_add(w * relu(v)[src], tgt) + x_t)/tau

Sharding: each of 8 cores owns 6250 target nodes x all 8 batches.  Per step
an AllGather exchanges r = relu(v); each core then processes its 200K
incoming edges locally:
  - partitions p = 16g + 2b + s:  g = source eighth (== source core),
    s = source half within eighth, b = batch.
  - per (g,s) "stream": edges with source in that sixteenth, sorted by
    local target; ap_gather fetches r[src] per stream (idx shared across
    the 16 partitions of GPSIMD group g; valid on matching-s partitions).
  - copy_predicated merges the two streams, multiply by per-edge weight
    (pre-scaled by DT/tau[tgt]), tensor_tensor_scan gives the running
    prefix sum; ap_gather samples it at per-target boundary positions;
    adjacent-difference yields per-stream partial syn.
  - one [128,8] matmul per 512-col chunk merges the 16 (g,s,b) partials
    per batch; PE transposes [8,128] blocks into the [128, 49*8] update
    layout; v update runs on DVE; relu on ACT.
All shapes/permutations are precomputed on host (numpy) from the fixed
graph; node space is padded to 8 * 6272 = 50176.
"""

import os
import sys
import time

os.environ.setdefault("JAX_COMPILATION_CACHE_DIR", "/tmp/jax_cache_gnn")
os.environ.setdefault("JAX_PERSISTENT_CACHE_MIN_COMPILE_TIME_SECS", "2")
os.environ.setdefault("JAX_PERSISTENT_CACHE_MIN_ENTRY_SIZE_BYTES", "0")

for _p in ("/opt/trn_rl_repo", "/root/.axon_site/_ro/trn_rl_repo"):
    if os.path.isdir(_p) and _p not in sys.path:
        sys.path.insert(0, _p)

import numpy as np

N_NODES = 50000
N_EDGES = 1_600_000
T = 50
B = 8
DT = 0.02

NC = 8  # cores
CORE_REAL = 6250  # real nodes per core
CORE_PAD = 6400  # padded nodes per core (= 50*128)
SLICE = 3200  # nodes per (g,s) sixteenth slice
TOTAL_PAD = NC * CORE_PAD  # 51200
NQ = 50  # 128-node windows per core
UPD_COLS = NQ * B  # 400

CH = 2048  # edge chunk (src gather size)
NCH = 7  # chunks per stream
STREAM = NCH * CH  # 14336 slots per (g,s) stream
RING = 4  # scan ring slots
RINGC = RING * CH  # 8192
TBC = 1024  # boundary targets per chunk
NTC = 7  # boundary chunks (7*1024 = 7168 >= 6272)
BOUND_T = NTC * TBC  # 7168 (>= CORE_PAD)

_CACHE = {}


def _wrap_idx_groups(idx_by_group):
    """idx_by_group: [8, N] int -> [128, N//16] int16 wrapped per group."""
    G, N = idx_by_group.shape
    assert G == 8 and N % 16 == 0
    out = np.empty((128, N // 16), dtype=np.int16)
    for g in range(8):
        out[16 * g : 16 * g + 16, :] = (
            idx_by_group[g].reshape(N // 16, 16).T.astype(np.int16)
        )
    return out


def _preprocess(x, bias, time_const, sign, syn_count, syn_strength,
                source_idx, target_idx):
    """Host-side graph compilation -> per-core input dicts."""
    tau = np.maximum(time_const.astype(np.float64), DT)
    BC = (DT / tau).astype(np.float64)  # per real node
    A = (1.0 - DT / tau).astype(np.float32)
    weight = (sign.astype(np.float64) * syn_count.astype(np.float64)
              * np.maximum(syn_strength.astype(np.float64), 0.0))

    src = source_idx.astype(np.int64)
    tgt = target_idx.astype(np.int64)
    # padded node ids
    def pid(n):
        return (n // CORE_REAL) * CORE_PAD + (n % CORE_REAL)

    spid = pid(src)
    tpid = pid(tgt)
    tcore = tpid // CORE_PAD
    tloc = tpid % CORE_PAD
    g = spid // CORE_PAD
    s = (spid % CORE_PAD) // SLICE
    sloc = spid % SLICE
    wprime = (weight * BC[tgt]).astype(np.float32)

    # global sort by (tcore, g, s, tloc)
    order = np.lexsort((tloc, s, g, tcore))
    spid_s, tcore_s, g_s, s_s = spid[order], tcore[order], g[order], s[order]
    sloc_s, tloc_s, w_s = sloc[order], tloc[order], wprime[order]
    key = ((tcore_s * 8 + g_s) * 2 + s_s)
    starts = np.searchsorted(key, np.arange(NC * 16), side="left")
    ends = np.searchsorted(key, np.arange(NC * 16), side="right")
    maxlen = int((ends - starts).max())
    assert maxlen + 1 <= STREAM, f"stream overflow: {maxlen + 1} > {STREAM}"

    per_core = []
    for c in range(NC):
        idx_streams = np.zeros((8, 2, STREAM), dtype=np.int16)
        w_streams = np.zeros((8, 2, STREAM), dtype=np.float32)
        bidx = np.zeros((8, 2, BOUND_T), dtype=np.int64)
        for gg in range(8):
            for ss in range(2):
                k = (c * 8 + gg) * 2 + ss
                a, b_ = int(starts[k]), int(ends[k])
                n = b_ - a
                # position 0 is a dummy edge (idx 0, w 0)
                idx_streams[gg, ss, 1 : n + 1] = sloc_s[a:b_]
                w_streams[gg, ss, 1 : n + 1] = w_s[a:b_]
                # boundary: last position with tloc <= t  (dummy at pos 0)
                cnt = np.searchsorted(tloc_s[a:b_], np.arange(BOUND_T),
                                      side="right")
                bidx[gg, ss] = cnt  # position = count (dummy shifts by 1)
        # ring-safety assert: boundaries for target-chunk tc are gathered
        # after edge chunk min(tc+1, NCH-1); ring holds RING chunks.
        for tc in range(NTC):
            ec = min(tc + 1, NCH - 1)
            lo = (ec + 1 - RING) * CH
            mx = int(bidx[:, :, tc * TBC : (tc + 1) * TBC].max())
            mn = int(bidx[:, :, tc * TBC : (tc + 1) * TBC].min())
            assert mx < (ec + 1) * CH, (tc, mx)
            assert mn >= lo or lo <= 0, (tc, mn, lo)

        rbidx = (bidx % RINGC).astype(np.int16)
        idxA = _wrap_idx_groups(idx_streams[:, 0, :])
        idxB = _wrap_idx_groups(idx_streams[:, 1, :])
        bidxA = _wrap_idx_groups(rbidx[:, 0, :])
        bidxB = _wrap_idx_groups(rbidx[:, 1, :])
        # weights in partition layout p = 16g + 2b + s
        wq = np.zeros((128, STREAM), dtype=np.float32)
        for gg in range(8):
            for ss in range(2):
                for bb in range(B):
                    wq[16 * gg + 2 * bb + ss] = w_streams[gg, ss]

        # per-core node constants / inputs in update layout [128, 392]
        n0 = c * CORE_REAL
        sl = slice(n0, n0 + CORE_REAL)

        def to_layout(vec_b_n):  # [B, CORE_PAD] -> [128, 392]
            return (vec_b_n.reshape(B, NQ, 128).transpose(2, 1, 0)
                    .reshape(128, UPD_COLS).astype(np.float32))

        Ap = np.zeros((B, CORE_PAD), dtype=np.float32)
        Ap[:, :CORE_REAL] = A[sl][None, :]
        v0p = np.zeros((B, CORE_PAD), dtype=np.float32)
        v0p[:, :CORE_REAL] = bias[sl][None, :]
        Tl = x.shape[0]
        xc = np.zeros((Tl, B, CORE_PAD), dtype=np.float32)
        xc[:, :, :CORE_REAL] = (
            BC[sl][None, None, :]
            * (x[:, :, sl].astype(np.float64) + bias[sl][None, None, :])
        ).astype(np.float32)
        xprime = (xc.reshape(Tl, B, NQ, 128).transpose(0, 3, 2, 1)
                  .reshape(Tl, 128, UPD_COLS))

        mask = np.zeros((128, CH), dtype=np.uint32)
        mask[1::2, :] = 1  # s=1 partitions (p odd)
        sel = np.zeros((128, 8), dtype=np.float32)
        for p in range(128):
            sel[p, (p % 16) // 2] = 1.0
        ident = np.eye(128, dtype=np.float32)

        per_core.append(dict(
            wq=wq, idxA=idxA, idxB=idxB, bidxA=bidxA, bidxB=bidxB,
            xprime=np.ascontiguousarray(xprime),
            Ad=to_layout(Ap), v0=to_layout(v0p),
            mask=mask, sel=sel, ident=ident,
        ))
    return per_core


def _build(T_steps, reps=1, tiny_x=False):
    import concourse.bacc as bacc
    import concourse.mybir as mybir
    import concourse.tile as tile

    dt = mybir.dt
    AF = mybir.ActivationFunctionType
    OP = mybir.AluOpType
    nc = bacc.Bacc("TRN2", target_bir_lowering=False, debug=False,
                   num_devices=NC)

    wq_d = nc.dram_tensor("wq", [128, STREAM], dt.float32, kind="ExternalInput")
    idxA_d = nc.dram_tensor("idxA", [128, STREAM // 16], dt.int16,
                            kind="ExternalInput")
    idxB_d = nc.dram_tensor("idxB", [128, STREAM // 16], dt.int16,
                            kind="ExternalInput")
    bidxA_d = nc.dram_tensor("bidxA", [128, BOUND_T // 16], dt.int16,
                             kind="ExternalInput")
    bidxB_d = nc.dram_tensor("bidxB", [128, BOUND_T // 16], dt.int16,
                             kind="ExternalInput")
    xprime_d = nc.dram_tensor("xprime",
                              [1 if tiny_x else T_steps, 128, UPD_COLS],
                              dt.float32, kind="ExternalInput")
    Ad_d = nc.dram_tensor("Ad", [128, UPD_COLS], dt.float32,
                          kind="ExternalInput")
    v0_d = nc.dram_tensor("v0", [128, UPD_COLS], dt.float32,
                          kind="ExternalInput")
    mask_d = nc.dram_tensor("mask", [128, CH], dt.uint32, kind="ExternalInput")
    sel_d = nc.dram_tensor("sel", [128, 8], dt.float32, kind="ExternalInput")
    ident_d = nc.dram_tensor("ident", [128, 128], dt.float32,
                             kind="ExternalInput")
    out_d = nc.dram_tensor("vs", [T_steps, 128, UPD_COLS], dt.float32,
                           kind="ExternalOutput")

    with tile.TileContext(nc) as tc:
        with (
            tc.tile_pool(name="sbuf", bufs=1) as pool,
            tc.tile_pool(name="psum", bufs=2, space="PSUM") as psum_pool,
            tc.tile_pool(name="dram", bufs=1, space="DRAM") as dram_pool,
        ):
            wq = pool.tile_from(wq_d[:])
            idxA = pool.tile_from(idxA_d[:])
            idxB = pool.tile_from(idxB_d[:])
            bidxA = pool.tile_from(bidxA_d[:])
            bidxB = pool.tile_from(bidxB_d[:])
            Ad = pool.tile_from(Ad_d[:])
            mask = pool.tile_from(mask_d[:])
            sel = pool.tile_from(sel_d[:])
            ident = pool.tile_from(ident_d[:])
            v = pool.tile_from(v0_d[:])

            r_sb = pool.tile([128, UPD_COLS], dt.float32)
            r_full = pool.tile([128, SLICE], dt.float32)
            bufA = pool.tile([128, CH], dt.float32)
            bufB = pool.tile([128, CH], dt.float32)
            ring = pool.tile([128, RINGC], dt.float32)
            bound = pool.tile([128, 1 + BOUND_T], dt.float32)
            bscr = pool.tile([128, TBC], dt.float32)
            xcur = pool.tile([128, UPD_COLS], dt.float32, tag="xq0")
            xnxt = pool.tile([128, UPD_COLS], dt.float32, tag="xq1")
            t1 = pool.tile([128, UPD_COLS], dt.float32)

            r_own = dram_pool.tile([B, CORE_PAD], dt.float32)
            r_all = dram_pool.tile([NC, B * CORE_PAD], dt.float32)

            nc.sync.dma_start(xcur[:], xprime_d[0])
            nc.vector.memset(ring[:], 0.0)

            xt = [xcur, xnxt]

            def full_loop():
                for t in range(T_steps):
                    # ---- halo exchange of r = relu(v) ----
                    nc.scalar.activation(r_sb[:], v[:], AF.Relu)
                    for bb in range(B):
                        nc.sync.dma_start(
                            r_own[bb : bb + 1, :].rearrange(
                                "o (q p) -> (o p) q", p=128),
                            r_sb[:, bb :: B],
                        )
                    nc.gpsimd.collective_compute(
                        "AllGather", OP.bypass,
                        replica_groups=[list(range(NC))],
                        ins=[r_own[:].opt()], outs=[r_all[:].opt()],
                    )
                    nc.sync.dma_start(
                        r_full[:],
                        r_all[:].rearrange("g (b s n) -> (g b s) n", b=B, s=2),
                    )
                    if t + 1 < T_steps:
                        nc.sync.dma_start(xt[(t + 1) % 2][:],
                                          xprime_d[0 if tiny_x else t + 1])

                    # ---- edge phase ----
                    for ec in range(NCH):
                        cs = slice(ec * CH, (ec + 1) * CH)
                        nc.gpsimd.ap_gather(bufA[:], r_full[:],
                                            idxA[:, ec * CH // 16 : (ec + 1) * CH // 16],
                                            channels=128, num_elems=SLICE, d=1,
                                            num_idxs=CH)
                        nc.gpsimd.ap_gather(bufB[:], r_full[:],
                                            idxB[:, ec * CH // 16 : (ec + 1) * CH // 16],
                                            channels=128, num_elems=SLICE, d=1,
                                            num_idxs=CH)
                        nc.vector.copy_predicated(bufA[:], mask[:], bufB[:])
                        nc.vector.tensor_mul(bufA[:], bufA[:], wq[:, cs])
                        rs = slice((ec % RING) * CH, (ec % RING + 1) * CH)
                        init = (0.0 if ec == 0 else
                                ring[:, ((ec - 1) % RING) * CH + CH - 1 :
                                     ((ec - 1) % RING) * CH + CH])
                        nc.vector.tensor_tensor_scan(
                            ring[:, rs], bufA[:], bufA[:], init,
                            op0=OP.add, op1=OP.bypass,
                        )
                        # boundary for target-chunk tc once its edges are scanned
                        tcs = []
                        if ec >= 2:
                            tcs.append(ec - 2)
                        if ec == NCH - 1:
                            tcs += [NCH - 2, NCH - 1]
                        for tci in tcs:
                            bs = slice(1 + tci * TBC, 1 + (tci + 1) * TBC)
                            nc.gpsimd.ap_gather(
                                bound[:, bs], ring[:],
                                bidxA[:, tci * TBC // 16 : (tci + 1) * TBC // 16],
                                channels=128, num_elems=RINGC, d=1, num_idxs=TBC)
                            nc.gpsimd.ap_gather(
                                bscr[:], ring[:],
                                bidxB[:, tci * TBC // 16 : (tci + 1) * TBC // 16],
                                channels=128, num_elems=RINGC, d=1, num_idxs=TBC)
                            nc.vector.copy_predicated(bound[:, bs],
                                                      mask[:, :TBC], bscr[:])

                    # ---- diff + batch merge + transpose to update layout ----
                    nc.vector.memset(bound[:, 0:1], 0.0)
                    nc.vector.tensor_tensor(
                        out=bound[:, 0:CORE_PAD],
                        in0=bound[:, 1 : CORE_PAD + 1],
                        in1=bound[:, 0:CORE_PAD],
                        op=OP.subtract,
                    )
                    psum2 = psum_pool.tile([128, UPD_COLS], dt.float32,
                                           space="PSUM", tag="upd")
                    nmm = (CORE_PAD + 511) // 512
                    for mc in range(nmm):
                        ncol = min(512, CORE_PAD - mc * 512)
                        ps = psum_pool.tile([8, 512], dt.float32, space="PSUM",
                                            tag="mm")
                        nc.tensor.matmul(ps[:, :ncol], sel[:],
                                         bound[:, mc * 512 : mc * 512 + ncol],
                                         start=True, stop=True)
                        nc.scalar.activation(
                            bound[:8, mc * 512 : mc * 512 + ncol], ps[:, :ncol],
                            AF.Copy)
                    for q in range(NQ):
                        nc.tensor.transpose(
                            psum2[:, q * 8 : (q + 1) * 8],
                            bound[:8, q * 128 : (q + 1) * 128],
                            ident[:8, :8],
                        )

                    # ---- update ----
                    nc.vector.tensor_tensor(t1[:], psum2[:], xt[t % 2][:],
                                            op=OP.add)
                    nc.vector.tensor_mul(v[:], v[:], Ad[:])
                    nc.vector.tensor_add(v[:], v[:], t1[:])
                    nc.sync.dma_start(out_d[t], v[:])


            if reps > 1:
                with tc.For_i(0, reps, 1) as _k:
                    nc.sync.dma_start(v[:], v0_d[:])
                    full_loop()
            else:
                full_loop()

    nc.compile()
    return nc


def _get_nc(T_steps):
    key = ("nc", T_steps)
    if key not in _CACHE:
        _CACHE[key] = _build(T_steps)
    return _CACHE[key]


def kernel(x, bias, time_const, sign, syn_count, syn_strength,
           source_idx, target_idx):
    from concourse.bass_utils import run_bass_kernel_spmd

    x = np.asarray(x, dtype=np.float32)
    bias = np.asarray(bias, dtype=np.float32)
    time_const = np.asarray(time_const, dtype=np.float32)
    sign = np.asarray(sign, dtype=np.float32)
    syn_count = np.asarray(syn_count, dtype=np.float32)
    syn_strength = np.asarray(syn_strength, dtype=np.float32)
    T_steps = x.shape[0]

    per_core = _preprocess(x, bias, time_const, sign, syn_count,
                           syn_strength, source_idx, target_idx)
    nc = _get_nc(T_steps)
    t0 = time.perf_counter()
    res = run_bass_kernel_spmd(nc, per_core, core_ids=list(range(NC)))
    t1 = time.perf_counter()
    print(f"[kernel] run_bass_kernel_spmd wall: {t1 - t0:.3f}s",
          file=sys.stderr)

    out = np.empty((T_steps, B, N_NODES), dtype=np.float32)
    for c in range(NC):
        vs = res.results[c]["vs"]  # [T, 128, 392]
        vbn = (vs.reshape(T_steps, 128, NQ, B).transpose(0, 3, 2, 1)
               .reshape(T_steps, B, CORE_PAD))
        out[:, :, c * CORE_REAL : (c + 1) * CORE_REAL] = vbn[:, :, :CORE_REAL]
    return out



# revision 6
# speedup vs baseline: 137.1267x; 137.1267x over previous
"""Trainium2 Bass kernel v2 for nn_Network_5772436046487 (gnn_message_passing).

Recurrence (T=50 steps, B=8, N=50000 nodes, E=1.6M edges):
    v' = v + DT*(-v + bias + scatter_add(w * relu(v)[src], tgt) + x_t)/tau

v1 -> v2 changes, driven by HW micro-benchmarks (ap_gather costs ~26ns per
index column regardless of instruction size; AllGather latency is only
~6us when chained; per-instruction overheads and padding are the rest):
  - 6 ap_gathers/step instead of 28: edge streams in 2 chunks of 6560
    (A+B streams), boundary sampling as 2 full gathers of 6416 from a
    full-prefix scan buffer (no ring; ~39K gathered columns/step total,
    down from 43K via tighter stream padding).
  - SBUF region aliasing: `scratch` [128,13120] holds edge-gather outputs
    AND boundary samples/merged partials; `scanbuf` [128,13120] holds
    weighted currents and the in-place prefix scan.
  - wq stored bf16 (halves its SBUF footprint; rel err ~2e-4).
  - Shared-address-space AllGather output (direct remote writes).
Per step: relu -> r staging DMAs -> AllGather -> r_full load -> 4 edge
gathers + pred/mul/scan per chunk -> 2 boundary gathers -> pred ->
adjacent-diff -> 13 sel-matmuls + ACT copies + 50 PE transposes ->
v update -> out DMA.  All shapes/permutations precomputed on host from
the fixed graph; node space padded to 8 * 6400 = 51200.
"""

import os
import sys
import time

os.environ.setdefault("JAX_COMPILATION_CACHE_DIR", "/tmp/jax_cache_gnn")
os.environ.setdefault("JAX_PERSISTENT_CACHE_MIN_COMPILE_TIME_SECS", "2")
os.environ.setdefault("JAX_PERSISTENT_CACHE_MIN_ENTRY_SIZE_BYTES", "0")

for _p in ("/opt/trn_rl_repo", "/root/.axon_site/_ro/trn_rl_repo"):
    if os.path.isdir(_p) and _p not in sys.path:
        sys.path.insert(0, _p)

import numpy as np

N_NODES = 50000
N_EDGES = 1_600_000
T = 50
B = 8
DT = 0.02

NC = 8  # cores
CORE_REAL = 6250  # real nodes per core
CORE_PAD = 6400  # padded nodes per core
SLICE = 3200  # nodes per (g,s) sixteenth slice
NQ = 50  # 128-node windows per core
UPD_COLS = NQ * B  # 400 cols, update layout [128, (q b)]

CH = 6560  # edge chunk (idx per gather)
NCH = 2
STREAM = NCH * CH  # 13120 slots per (g,s) stream (data max: 13103)
BOUND_T = 6416  # boundary samples per stream (1 dummy + 6400 + pad)

_CACHE = {}


def _wrap_idx_groups(idx_by_group):
    """idx_by_group: [8, N] int -> [128, N//16] int16 wrapped per group."""
    G, N = idx_by_group.shape
    assert G == 8 and N % 16 == 0
    out = np.empty((128, N // 16), dtype=np.int16)
    for g in range(8):
        out[16 * g : 16 * g + 16, :] = (
            idx_by_group[g].reshape(N // 16, 16).T.astype(np.int16)
        )
    return out


def _preprocess(x, bias, time_const, sign, syn_count, syn_strength,
                source_idx, target_idx):
    """Host-side graph compilation -> per-core input dicts."""
    from concourse import mybir

    bf16 = mybir.dt.np(mybir.dt.bfloat16)
    tau = np.maximum(time_const.astype(np.float64), DT)
    BC = (DT / tau).astype(np.float64)  # per real node
    A = (1.0 - DT / tau).astype(np.float32)
    weight = (sign.astype(np.float64) * syn_count.astype(np.float64)
              * np.maximum(syn_strength.astype(np.float64), 0.0))

    src = source_idx.astype(np.int64)
    tgt = target_idx.astype(np.int64)

    def pid(n):
        return (n // CORE_REAL) * CORE_PAD + (n % CORE_REAL)

    spid = pid(src)
    tpid = pid(tgt)
    tcore = tpid // CORE_PAD
    tloc = tpid % CORE_PAD
    g = spid // CORE_PAD
    s = (spid % CORE_PAD) // SLICE
    sloc = spid % SLICE
    wprime = (weight * BC[tgt]).astype(np.float32)

    order = np.lexsort((tloc, s, g, tcore))
    spid_s, tcore_s, g_s, s_s = spid[order], tcore[order], g[order], s[order]
    sloc_s, tloc_s, w_s = sloc[order], tloc[order], wprime[order]
    key = ((tcore_s * 8 + g_s) * 2 + s_s)
    starts = np.searchsorted(key, np.arange(NC * 16), side="left")
    ends = np.searchsorted(key, np.arange(NC * 16), side="right")
    maxlen = int((ends - starts).max())
    assert maxlen + 1 <= STREAM, f"stream overflow: {maxlen + 1} > {STREAM}"

    def to_upd_layout(vec_b_n):  # [B, CORE_PAD] -> [128, 400], node = q*128+p
        return (vec_b_n.reshape(B, NQ, 128).transpose(2, 1, 0)
                .reshape(128, UPD_COLS).astype(np.float32))

    per_core = []
    for c in range(NC):
        idx_streams = np.zeros((8, 2, STREAM), dtype=np.int16)
        w_streams = np.zeros((8, 2, STREAM), dtype=np.float32)
        bidx = np.zeros((8, 2, BOUND_T), dtype=np.int64)
        for gg in range(8):
            for ss in range(2):
                k = (c * 8 + gg) * 2 + ss
                a, b_ = int(starts[k]), int(ends[k])
                n = b_ - a
                # position 0 is a dummy edge (idx 0, w 0)
                idx_streams[gg, ss, 1 : n + 1] = sloc_s[a:b_]
                w_streams[gg, ss, 1 : n + 1] = w_s[a:b_]
                # sample positions: col 0 = dummy pos 0; col 1+t = cnt(t)
                cnt = np.searchsorted(tloc_s[a:b_], np.arange(CORE_PAD),
                                      side="right")
                bidx[gg, ss, 1 : 1 + CORE_PAD] = cnt
        assert int(bidx.max()) < STREAM

        idxA = _wrap_idx_groups(idx_streams[:, 0, :])
        idxB = _wrap_idx_groups(idx_streams[:, 1, :])
        bidxA = _wrap_idx_groups(bidx[:, 0, :].astype(np.int16))
        bidxB = _wrap_idx_groups(bidx[:, 1, :].astype(np.int16))
        # weights in partition layout p = 16g + 2b + s
        wq = np.zeros((128, STREAM), dtype=np.float32)
        for gg in range(8):
            for ss in range(2):
                for bb in range(B):
                    wq[16 * gg + 2 * bb + ss] = w_streams[gg, ss]
        wq = wq.astype(bf16)

        n0 = c * CORE_REAL
        sl = slice(n0, n0 + CORE_REAL)

        Ap = np.zeros((B, CORE_PAD), dtype=np.float32)
        Ap[:, :CORE_REAL] = A[sl][None, :]
        v0p = np.zeros((B, CORE_PAD), dtype=np.float32)
        v0p[:, :CORE_REAL] = bias[sl][None, :]
        Tl = x.shape[0]
        xc = np.zeros((Tl, B, CORE_PAD), dtype=np.float32)
        xc[:, :, :CORE_REAL] = (
            BC[sl][None, None, :]
            * (x[:, :, sl].astype(np.float64) + bias[sl][None, None, :])
        ).astype(np.float32)
        xprime = (xc.reshape(Tl, B, NQ, 128).transpose(0, 3, 2, 1)
                  .reshape(Tl, 128, UPD_COLS))

        sel = np.zeros((128, 8), dtype=np.float32)
        for p in range(128):
            sel[p, (p % 16) // 2] = 1.0
        mask = np.zeros((128, 8), dtype=np.uint32)
        mask[1::2, :] = 1  # s=1 partitions (p odd)
        ident = np.eye(128, dtype=np.float32)

        per_core.append(dict(
            wq=wq, idxA=idxA, idxB=idxB, bidxA=bidxA, bidxB=bidxB,
            xprime=np.ascontiguousarray(xprime),
            Ad=to_upd_layout(Ap), v0=to_upd_layout(v0p),
            mask=mask, sel=sel, ident=ident,
        ))
    return per_core


def _build(T_steps, tiny_x=False, shared_out=True):
    import concourse.bacc as bacc
    import concourse.mybir as mybir
    import concourse.tile as tile

    dt = mybir.dt
    AF = mybir.ActivationFunctionType
    OP = mybir.AluOpType
    nc = bacc.Bacc("TRN2", target_bir_lowering=False, debug=False,
                   num_devices=NC)

    wq_d = nc.dram_tensor("wq", [128, STREAM], dt.bfloat16,
                          kind="ExternalInput")
    idxA_d = nc.dram_tensor("idxA", [128, STREAM // 16], dt.int16,
                            kind="ExternalInput")
    idxB_d = nc.dram_tensor("idxB", [128, STREAM // 16], dt.int16,
                            kind="ExternalInput")
    bidxA_d = nc.dram_tensor("bidxA", [128, BOUND_T // 16], dt.int16,
                             kind="ExternalInput")
    bidxB_d = nc.dram_tensor("bidxB", [128, BOUND_T // 16], dt.int16,
                             kind="ExternalInput")
    xprime_d = nc.dram_tensor("xprime",
                              [1 if tiny_x else T_steps, 128, UPD_COLS],
                              dt.float32, kind="ExternalInput")
    Ad_d = nc.dram_tensor("Ad", [128, UPD_COLS], dt.float32,
                          kind="ExternalInput")
    v0_d = nc.dram_tensor("v0", [128, UPD_COLS], dt.float32,
                          kind="ExternalInput")
    mask_d = nc.dram_tensor("mask", [128, 8], dt.uint32, kind="ExternalInput")
    sel_d = nc.dram_tensor("sel", [128, 8], dt.float32, kind="ExternalInput")
    ident_d = nc.dram_tensor("ident", [128, 128], dt.float32,
                             kind="ExternalInput")
    out_d = nc.dram_tensor("vs", [T_steps, 128, UPD_COLS], dt.float32,
                           kind="ExternalOutput")
    r_all_d = nc.dram_tensor("r_all_sh", [NC, B * CORE_PAD], dt.float32,
                             addr_space="Shared" if shared_out else "Local")

    with tile.TileContext(nc) as tc:
        with (
            tc.tile_pool(name="sbuf", bufs=1) as pool,
            tc.tile_pool(name="psum", bufs=2, space="PSUM") as psum_pool,
            tc.tile_pool(name="dram", bufs=1, space="DRAM") as dram_pool,
        ):
            wq = pool.tile_from(wq_d[:])
            idxA = pool.tile_from(idxA_d[:])
            idxB = pool.tile_from(idxB_d[:])
            bidxA = pool.tile_from(bidxA_d[:])
            bidxB = pool.tile_from(bidxB_d[:])
            Ad = pool.tile_from(Ad_d[:])
            mask8 = pool.tile_from(mask_d[:])
            sel = pool.tile_from(sel_d[:])
            ident = pool.tile_from(ident_d[:])
            v = pool.tile_from(v0_d[:])

            r_sb = pool.tile([128, UPD_COLS], dt.float32)
            r_full = pool.tile([128, SLICE], dt.float32)
            scratch = pool.tile([128, STREAM], dt.float32)
            scanbuf = pool.tile([128, STREAM], dt.float32)
            xcur = pool.tile([128, UPD_COLS], dt.float32, tag="xq0")
            xnxt = pool.tile([128, UPD_COLS], dt.float32, tag="xq1")
            t1 = pool.tile([128, UPD_COLS], dt.float32)

            r_own = dram_pool.tile([B, CORE_PAD], dt.float32)

            nc.sync.dma_start(xcur[:], xprime_d[0])

            xt = [xcur, xnxt]
            maskCH = mask8[:, 0:1].broadcast_to([128, CH])
            maskBT = mask8[:, 0:1].broadcast_to([128, BOUND_T])

            for t in range(T_steps):
                # ---- halo exchange of r = relu(v) ----
                nc.scalar.activation(r_sb[:], v[:], AF.Relu)
                for bb in range(B):
                    nc.sync.dma_start(
                        r_own[bb : bb + 1, :].rearrange(
                            "o (q p) -> (o p) q", p=128),
                        r_sb[:, bb :: B],
                    )
                nc.gpsimd.collective_compute(
                    "AllGather", OP.bypass,
                    replica_groups=[list(range(NC))],
                    ins=[r_own[:].opt()], outs=[r_all_d[:].opt()],
                )
                nc.sync.dma_start(
                    r_full[:],
                    r_all_d[:].rearrange("g (b s n) -> (g b s) n", b=B, s=2),
                )
                if t + 1 < T_steps:
                    nc.sync.dma_start(xt[(t + 1) % 2][:],
                                      xprime_d[0 if tiny_x else t + 1])

                # ---- edge phase: 2 chunks x (gather A, gather B) ----
                for ec in range(NCH):
                    lo = slice(ec * CH, (ec + 1) * CH)
                    nc.gpsimd.ap_gather(
                        scratch[:, 0:CH] if ec == 0 else scanbuf[:, CH:],
                        r_full[:],
                        idxA[:, ec * CH // 16 : (ec + 1) * CH // 16],
                        channels=128, num_elems=SLICE, d=1, num_idxs=CH)
                    nc.gpsimd.ap_gather(
                        scratch[:, CH:], r_full[:],
                        idxB[:, ec * CH // 16 : (ec + 1) * CH // 16],
                        channels=128, num_elems=SLICE, d=1, num_idxs=CH)
                    if ec == 0:
                        # merge B into A (odd partitions), weight, scan
                        nc.vector.copy_predicated(scratch[:, 0:CH], maskCH,
                                                  scratch[:, CH:])
                        nc.vector.tensor_mul(scratch[:, 0:CH],
                                             scratch[:, 0:CH], wq[:, lo])
                        nc.vector.tensor_tensor_scan(
                            scanbuf[:, 0:CH], scratch[:, 0:CH],
                            scratch[:, 0:CH], 0.0,
                            op0=OP.add, op1=OP.bypass)
                    else:
                        nc.vector.copy_predicated(scanbuf[:, CH:], maskCH,
                                                  scratch[:, CH:])
                        nc.vector.tensor_mul(scanbuf[:, CH:],
                                             scanbuf[:, CH:], wq[:, lo])
                        nc.vector.tensor_tensor_scan(
                            scanbuf[:, CH:], scanbuf[:, CH:],
                            scanbuf[:, CH:],
                            scanbuf[:, CH - 1 : CH],
                            op0=OP.add, op1=OP.bypass)

                # ---- boundary sampling: 2 gathers of BOUND_T ----
                nc.gpsimd.ap_gather(
                    scratch[:, 0:BOUND_T], scanbuf[:],
                    bidxA[:], channels=128, num_elems=STREAM, d=1,
                    num_idxs=BOUND_T)
                nc.gpsimd.ap_gather(
                    scratch[:, CH : CH + BOUND_T], scanbuf[:],
                    bidxB[:], channels=128, num_elems=STREAM, d=1,
                    num_idxs=BOUND_T)
                nc.vector.copy_predicated(scratch[:, 0:BOUND_T], maskBT,
                                          scratch[:, CH : CH + BOUND_T])
                # adjacent difference -> per-target partials at cols 0..6400
                nc.vector.tensor_tensor(
                    out=scratch[:, 0:CORE_PAD],
                    in0=scratch[:, 1 : CORE_PAD + 1],
                    in1=scratch[:, 0:CORE_PAD],
                    op=OP.subtract,
                )

                # ---- merge 16 partials per batch, then transpose to (q b)
                psum2 = psum_pool.tile([128, UPD_COLS], dt.float32,
                                       space="PSUM", tag="upd")
                nmm = (CORE_PAD + 511) // 512
                ms = CH  # merged [8, 6400] parked in scratch cols CH..CH+6400
                for mc in range(nmm):
                    ncol = min(512, CORE_PAD - mc * 512)
                    ps = psum_pool.tile([8, 512], dt.float32, space="PSUM",
                                        tag="mm")
                    nc.tensor.matmul(ps[:, :ncol], sel[:],
                                     scratch[:, mc * 512 : mc * 512 + ncol],
                                     start=True, stop=True)
                    nc.scalar.activation(
                        scratch[:8, ms + mc * 512 : ms + mc * 512 + ncol],
                        ps[:, :ncol], AF.Copy)
                for q in range(NQ):
                    nc.tensor.transpose(
                        psum2[:, q * 8 : (q + 1) * 8],
                        scratch[:8, ms + q * 128 : ms + (q + 1) * 128],
                        ident[:8, :8])

                # ---- update ----
                nc.vector.tensor_tensor(t1[:], psum2[:], xt[t % 2][:],
                                        op=OP.add)
                nc.vector.tensor_mul(v[:], v[:], Ad[:])
                nc.vector.tensor_add(v[:], v[:], t1[:])
                nc.sync.dma_start(out_d[t], v[:])

    nc.compile()
    return nc


def _get_nc(T_steps):
    key = ("nc", T_steps)
    if key not in _CACHE:
        _CACHE[key] = _build(T_steps)
    return _CACHE[key]


def kernel(x, bias, time_const, sign, syn_count, syn_strength,
           source_idx, target_idx):
    from concourse.bass_utils import run_bass_kernel_spmd

    x = np.asarray(x, dtype=np.float32)
    bias = np.asarray(bias, dtype=np.float32)
    time_const = np.asarray(time_const, dtype=np.float32)
    sign = np.asarray(sign, dtype=np.float32)
    syn_count = np.asarray(syn_count, dtype=np.float32)
    syn_strength = np.asarray(syn_strength, dtype=np.float32)
    T_steps = x.shape[0]

    per_core = _preprocess(x, bias, time_const, sign, syn_count,
                           syn_strength, source_idx, target_idx)
    nc = _get_nc(T_steps)
    t0 = time.perf_counter()
    res = run_bass_kernel_spmd(nc, per_core, core_ids=list(range(NC)))
    t1 = time.perf_counter()
    print(f"[kernel] run_bass_kernel_spmd wall: {t1 - t0:.3f}s",
          file=sys.stderr)

    out = np.empty((T_steps, B, N_NODES), dtype=np.float32)
    for c in range(NC):
        vs = res.results[c]["vs"]  # [T, 128, 400], node = q*128+p
        vbn = (vs.reshape(T_steps, 128, NQ, B).transpose(0, 3, 2, 1)
               .reshape(T_steps, B, CORE_PAD))
        out[:, :, c * CORE_REAL : (c + 1) * CORE_REAL] = vbn[:, :, :CORE_REAL]
    return out
